# revision 46
# baseline (speedup 1.0000x reference)
"""Trainium2 Bass kernel for nn_Attn_128849019074 (sparse_attention).

reference:
    energy = einsum("lbd,ed->lbe", enc, W) + b        # [L,B,D] huge matmul
    scores = einsum("lbd,bd->lb", energy, hidden)     # [L,B]
    out    = log_softmax(scores, axis=1)[None, None]  # [1,1,L,B]

Algebraic rewrite (linearity):
    scores[l,b] = enc[l,b,:] . v[b,:] + c[b]
    with v = hidden @ W   ([B,D], tiny)  and  c = hidden @ b  ([B]).

This turns a 137-GMAC matmul into a single streaming pass over
encoder_outputs -> memory bound.  All streamed operands are cast to
bf16 on the host (rel err ~3e-3, tolerance 2e-2), halving HBM traffic
(enc 16.8 MB + W 8.4 MB per core).

Distribution: shard over L (128 timesteps per core).  The dim=1
log-softmax is over B=32, fully local per l -> no collectives.  W is
replicated (a v-AllGather's ~20-75 us trigger latency exceeds the
23 us bf16 W read).

Compute mapping: the multiply-reduce runs on the idle *TensorEngine*
(DVE tensor ops measure ~2.2 us per [128,2048] on HW -- far too slow
to keep up with the DMA stream).  enc is host-transposed so the
contraction dim d sits on partitions:

  - enc tile j [128, 16, 512] bf16: partition p, free (c, 32*lg+b) =
    enc[l = 16j+lg, b, 128c+p].
  - T_ps[b', col] += vT[:, c, b'].T @ et[:, c, :]   (16 chunk matmuls
    per tile, f32 PSUM accumulation = the full d-dot product).
  - diag extract: masked = T_ps * mask (DVE; mask[b', 32lg+b] = b'==b),
    then ones-matmul collapses partitions: sps[j%4, col] = sum_b' masked.
  - epilogue per 4-block half: +c, max, exp, ln on [4, (16, 32)] rows,
    one contiguous out DMA.  First half hidden under the stream.
"""

import os
import sys

sys.path.insert(0, "/opt/trn_rl_repo")

import numpy as np

L = 1024
B = 32
D = 2048
NCORES = 8
L_LOC = L // NCORES          # 128 timesteps per core
NBLK = 8                     # score blocks (PSUM [32, 512] each, 16 l's)
LG = L_LOC // NBLK           # 16 l's per block
NCH = D // 128               # 16 contraction chunks

_CACHE: dict = {}
last_results = None          # BassKernelResults from the most recent run


def _split_drain_waits(nc):
    """Walrus rejects Drain instructions carrying many sync waits ("Too many
    sync wait commands").  Tile's kernel-tail drain waits on every live
    semaphore lane at once; split it into a chain of single-wait drains."""
    import concourse.mybir as mybir

    for bb in nc.main_func.blocks:
        idx = 0
        while idx < len(bb.instructions):
            inst = bb.instructions[idx]
            if (
                isinstance(inst, mybir.InstDrain)
                and inst.sync_info is not None
                and len(inst.sync_info.on_wait or []) > 1
            ):
                waits = list(inst.sync_info.on_wait)
                spill, keep = waits[:-1], waits[-1:]
                new_insts = []
                for j, w in enumerate(spill):
                    x = mybir.InstDrain(name=f"{inst.name}_w{j}", ins=[], outs=[])
                    x.engine = inst.engine
                    x.sync_info = mybir.SyncInfo(on_wait=[w], on_update=[])
                    x.debug = inst.debug
                    nc.register_instruction(x)
                    new_insts.append(x)
                inst.sync_info = mybir.SyncInfo(
                    on_wait=keep, on_update=list(inst.sync_info.on_update or [])
                )
                bb.instructions[idx:idx] = new_insts
                idx += len(new_insts)
            idx += 1


def build_program():
    """Build (once) the SPMD Bass program shared by all 8 cores."""
    if "nc" in _CACHE:
        return _CACHE["nc"]

    import concourse.bacc as bacc
    import concourse.mybir as mybir
    import concourse.tile as tile

    f32 = mybir.dt.float32
    f16 = mybir.dt.float16
    bf16 = mybir.dt.bfloat16
    Alu = mybir.AluOpType
    Act = mybir.ActivationFunctionType

    nc = bacc.Bacc(
        "TRN2", target_bir_lowering=False, debug=False, num_devices=NCORES
    )

    enc = nc.dram_tensor(
        "enc", [NBLK * 128, NCH * 512], bf16, kind="ExternalInput"
    ).ap()
    # hbtt[p, 33c + j] = [hidden; b]^T[128c + p, j] — pre-tiled on the host so
    # the DMA is one contiguous run per partition.
    hbtt = nc.dram_tensor(
        "hbtt", [128, 16 * (B + 1)], bf16, kind="ExternalInput"
    ).ap()
    wfull = nc.dram_tensor("wfull", [D, D], bf16, kind="ExternalInput").ap()
    out = nc.dram_tensor("out", [L_LOC, B], f32, kind="ExternalOutput").ap()
    ident32 = nc.inline_tensor(np.eye(B, dtype=np.float32), "ident32").ap()
    # fp16 diag-collapse path: single-pass PE matmuls (f32 is LOW_HIGH 4x
    # slower); fp16's 10 mantissa bits keep the score error ~5e-4 relative.
    ones32 = nc.inline_tensor(np.ones((B, 1), dtype=np.float16), "ones32").ap()
    # ohmat[:, 4*jj + m] = 1 if m == jj — row-select for the diag collapse
    oh_np = np.zeros((B, 16), dtype=np.float16)
    for jj in range(4):
        oh_np[:, 4 * jj + jj] = 1.0
    ohmat = nc.inline_tensor(np.ascontiguousarray(oh_np), "ohmat").ap()
    # mask[b', 32*lg + b] = 1 if b' == b — diagonal-extraction mask
    mask_np = np.tile(np.eye(B, dtype=np.float16), (1, LG))
    mask = nc.inline_tensor(np.ascontiguousarray(mask_np), "mask").ap()

    with tile.TileContext(nc) as tc:
        with (
            tc.tile_pool(name="pers", bufs=1) as pers,
            tc.tile_pool(name="encp", bufs=5) as encp,
            tc.tile_pool(name="wp", bufs=4) as wp,
            tc.tile_pool(name="mkp", bufs=2) as mkp,
            tc.tile_pool(name="psp", bufs=1, space="PSUM") as psp,
            tc.tile_pool(name="tpsp", bufs=2, space="PSUM") as tpsp,
        ):
            # ---------- phase 1: v = hidden @ W and c = hidden @ b on PE ----
            # All small/const loads go on the *scalar* HWDGE queue so the
            # sync ring's very first descriptor is W chunk 0 — every 0.6 us
            # of issue ahead of W delays the whole stream.
            hbt_sb = pers.tile([128, 16 * (B + 1)], bf16)
            nc.scalar.dma_start(hbt_sb[:, :], hbtt[:, :])
            mask_sb = pers.tile([B, LG * B], f16)
            nc.scalar.dma_start(mask_sb[:, :], mask[:, :])
            ident_sb = pers.tile([B, B], f32)
            nc.scalar.dma_start(ident_sb[:, :], ident32[:, :])
            ones_sb = pers.tile([B, 1], f16)
            nc.scalar.dma_start(ones_sb[:, :], ones32[:, :])
            oh_sb = pers.tile([B, 16], f16)
            nc.scalar.dma_start(oh_sb[:, :], ohmat[:, :])

            # PE warm-up: ~4 us of back-to-back matmuls flips the HAM clock
            # gate from 1.2 to 2.4 GHz before the W-chunk matmuls begin.
            warm_ps = psp.tile([128, 512], f32, tag="big")
            for i in range(12):
                nc.tensor.matmul(
                    warm_ps[:, :], hbt_sb[:, 0:128], hbt_sb[:, 0:512]
                )
            warm_junk = pers.tile([1, 1], f32)
            nc.vector.tensor_copy(warm_junk[:, :], warm_ps[0:1, 0:1])

            v_ps = psp.tile([B, D], f32, tag="big")
            c_ps = psp.tile([B, 1], f32, tag="sps1")
            last_w_dma = None
            # W as 8 mid-size DMAs (1.05 MB): few enough that the HWDGE ring
            # is never issue-bound, small enough that the PE's v-matmul
            # bursts (~2.4 us) recur faster than the HAM idle window (3.4 us)
            # so the clock gate stays at 2.4 GHz through the whole W phase.
            for q in range(8):
                wc = wp.tile([128, 2, D], bf16, tag="wc")
                last_w_dma = nc.sync.dma_start(
                    wc[:, :, :],
                    wfull[256 * q : 256 * (q + 1), :].rearrange(
                        "(cc p) d -> p cc d", p=128
                    ),
                )
                for cc in range(2):
                    c = 2 * q + cc
                    lhs = hbt_sb[:, (B + 1) * c : (B + 1) * c + B]
                    for n in range(4):
                        nc.tensor.matmul(
                            v_ps[:, 512 * n : 512 * (n + 1)],
                            lhs,
                            wc[:, cc, 512 * n : 512 * (n + 1)],
                            start=(c == 0),
                            stop=(c == 15),
                        )
            for c in range(16):
                lhs = hbt_sb[:, (B + 1) * c : (B + 1) * c + B]
                rhs = hbt_sb[:, (B + 1) * c + B : (B + 1) * (c + 1)]
                nc.tensor.matmul(
                    c_ps[:, :], lhs, rhs, start=(c == 0), stop=(c == 15)
                )

            # ---------- phase 2: vT (PE transpose) + c replication ----------
            vsb = pers.tile([B, D], f32)
            nc.vector.tensor_copy(vsb[:, :], v_ps[:, :])
            vtr_ps = psp.tile([128, 512], f32, tag="sps0")
            for c in range(16):
                nc.tensor.transpose(
                    vtr_ps[:, B * c : B * (c + 1)],
                    vsb[:, 128 * c : 128 * (c + 1)],
                    ident_sb[:, :],
                )
            vT = pers.tile([128, NCH * B], bf16)  # [128, c, b'] = v[b', 128c+p]
            nc.vector.tensor_copy(vT[:, :], vtr_ps[:, :])
            # Re-warm the PE right before the stream: the v-phase tail can
            # leave the HAM clock gate cold, and a cold PE (1.2 GHz) cannot
            # keep pace with the enc stream, building an unrecoverable
            # backlog.  ~1.7 us of dummies guarantees a 2.4 GHz entry.
            wps2 = psp.tile([128, 512], f32, tag="big", name="wps2")
            for i in range(6):
                nc.tensor.matmul(wps2[:, :], hbt_sb[:, 0:128], hbt_sb[:, 0:512])

            # crep[jrow, 32*lg + b] = c[b]  (same for every jrow)
            c_sb = pers.tile([B, 1], f32)
            nc.vector.tensor_copy(c_sb[:, :], c_ps[:, :])
            cmask = pers.tile([B, LG * B], f16)
            nc.vector.tensor_scalar_mul(cmask[:, :], mask_sb[:, :], c_sb[:, 0:1])
            ones4 = pers.tile([B, 4], f16)
            for n in range(4):
                nc.vector.tensor_copy(ones4[:, n : n + 1], ones_sb[:, :])
            crep_ps = tpsp.tile([4, LG * B], f32, tag="T")
            nc.tensor.matmul(crep_ps[:, :], ones4[:, :], cmask[:, :])
            crep = pers.tile([4, LG * B], f32)
            nc.vector.tensor_copy(crep[:, :], crep_ps[:, :])
            tjunk = pers.tile([1, 2], f32)

            # ---------- phase 3: stream enc; multiply-reduce on the PE ------
            sps = [
                psp.tile([4, LG * B], f32, tag="sps0", name="sps0"),
                psp.tile([4, LG * B], f32, tag="sps1", name="sps1"),
            ]
            o_tiles = []
            for j in range(NBLK):
                # The last tile arrives as two half-DMAs so its first 8 chunk
                # matmuls overlap the second half's transfer (shorter tail).
                if j < NBLK - 1:
                    et = encp.tile([128, NCH, 512], bf16, tag="et")
                    parts = [(et, 0, NCH, nc.sync.dma_start(
                        et[:, :, :],
                        enc[128 * j : 128 * (j + 1), :].rearrange(
                            "p (c w) -> p c w", w=512
                        ),
                    ))]
                else:
                    eta = encp.tile([128, NCH // 2, 512], bf16, tag="et7a", bufs=1)
                    etb = encp.tile([128, NCH // 2, 512], bf16, tag="et7b", bufs=1)
                    row = enc[128 * j : 128 * (j + 1), :].rearrange(
                        "p (c w) -> p c w", w=512
                    )
                    parts = [
                        (eta, 0, NCH // 2,
                         nc.sync.dma_start(eta[:, :, :], row[:, 0 : NCH // 2, :])),
                        (etb, NCH // 2, NCH,
                         nc.sync.dma_start(etb[:, :, :], row[:, NCH // 2 :, :])),
                    ]
                # Keep the HWDGE FIFO ring W-first: vT must be ready early so
                # the PE can drain enc tiles as they land.
                for _, _, _, dma in parts:
                    tile.add_dep_helper(
                        dma.ins,
                        last_w_dma.ins,
                        sync=False,
                        reason="enc stream after W (W-first DMA ordering)",
                    )
                t_ps = tpsp.tile([B, LG * B], f32, tag="T")
                for part, c0, c1, _ in parts:
                    for c in range(c0, c1):
                        nc.tensor.matmul(
                            t_ps[:, :],
                            vT[:, B * c : B * (c + 1)],
                            part[:, c - c0, :],
                            start=(c == 0),
                            stop=(c == NCH - 1),
                        )
                masked = mkp.tile([B, LG * B], f16, tag="mk")
                nc.vector.tensor_mul(masked[:, :], t_ps[:, :], mask_sb[:, :])
                # Software-pipeline the diag collapse: block j-1's oh-matmul
                # is emitted AFTER tile j's chunk matmuls, so the PE never
                # stalls waiting for the DVE mask of the tile it just did.
                if pending is not None:
                    emit_diag(*pending)
                pending = (j, masked)
            emit_diag(*pending)

    nc.compile()
    _split_drain_waits(nc)
    _CACHE["nc"] = nc
    return nc


def _unreachable_epilogue_block():
    if True:
        raise AssertionError
    # (dead code placeholder, replaced below)
    if False:
                if j % 4 == 3:
                    h = j // 4
                    ssb = pers.tile([4, LG * B], f32, name=f"ssb{h}")
                    nc.vector.tensor_add(ssb[:, :], sps[h][:, :], crep[:, :])
                    mneg = pers.tile([4, LG], f32, name=f"mneg{h}")
                    nc.vector.tensor_reduce(
                        mneg[:, :],
                        ssb.rearrange("p (lg b) -> p lg b", b=B),
                        axis=mybir.AxisListType.X,
                        op=Alu.max,
                        negate=True,
                    )
                    # The two per-lg scalar chains are split DVE/ACT so the
                    # serial 16-op chain becomes two concurrent ~10/6 chains.
                    sm = pers.tile([4, LG * B], f32, name=f"sm{h}")
                    for g in range(LG):
                        if g < 10:
                            nc.vector.tensor_scalar_add(
                                sm[:, B * g : B * (g + 1)],
                                ssb[:, B * g : B * (g + 1)],
                                mneg[:, g : g + 1],
                            )
                        else:
                            nc.scalar.activation(
                                sm[:, B * g : B * (g + 1)],
                                ssb[:, B * g : B * (g + 1)],
                                Act.Identity,
                                bias=mneg[:, g : g + 1],
                            )
                    es = pers.tile([4, LG * B], f32, name=f"es{h}")
                    nc.scalar.activation(es[:, :], sm[:, :], Act.Exp)
                    s16 = pers.tile([4, LG], f32, name=f"s16{h}")
                    nc.vector.tensor_reduce(
                        s16[:, :],
                        es.rearrange("p (lg b) -> p lg b", b=B),
                        axis=mybir.AxisListType.X,
                        op=Alu.add,
                    )
                    ln16 = pers.tile([4, LG], f32, name=f"ln16{h}")
                    nc.scalar.activation(ln16[:, :], s16[:, :], Act.Ln)
                    ln16n = pers.tile([4, LG], f32, name=f"ln16n{h}")
                    nc.vector.tensor_scalar_mul(ln16n[:, :], ln16[:, :], -1.0)
                    o = pers.tile([4, LG * B], f32, name=f"o{h}")
                    for g in range(LG):
                        if g < 10:
                            nc.vector.tensor_scalar_sub(
                                o[:, B * g : B * (g + 1)],
                                sm[:, B * g : B * (g + 1)],
                                ln16[:, g : g + 1],
                            )
                        else:
                            nc.scalar.activation(
                                o[:, B * g : B * (g + 1)],
                                sm[:, B * g : B * (g + 1)],
                                Act.Identity,
                                bias=ln16n[:, g : g + 1],
                            )
                    o_tiles.append(o)
                    # out rows l = 64h + 16*jrow + lg ; one contiguous DMA
                    # per half on the scalar HWDGE queue (keeps the sync
                    # queue free for the enc stream).
                    out_h = out.rearrange("(h j lgb) b -> h j (lgb b)", h=2, j=4)
                    nc.scalar.dma_start(out_h[h, :, :], o[:, :])

    nc.compile()
    _split_drain_waits(nc)
    _CACHE["nc"] = nc
    return nc


def make_in_maps(hidden, encoder_outputs, W, b):
    import ml_dtypes

    bf16 = ml_dtypes.bfloat16
    hidden = np.asarray(hidden, dtype=np.float32)
    enc16 = np.asarray(encoder_outputs, dtype=np.float32).astype(bf16)
    W16 = np.ascontiguousarray(np.asarray(W, dtype=np.float32).astype(bf16))
    b_ = np.asarray(b, dtype=np.float32)
    hb = np.concatenate([hidden, b_[None, :]], axis=0)  # [33, D]
    # hbtt[p, 33c + j] = hb[j, 128c + p] — the SBUF tile layout, host-built
    hbtt16 = np.ascontiguousarray(
        hb.T.reshape(16, 128, B + 1)
        .transpose(1, 0, 2)
        .reshape(128, 16 * (B + 1))
        .astype(bf16)
    )
    in_maps = []
    for k in range(NCORES):
        # tile j, partition p, free (c, 32*lg+b) <- enc[l=16j+lg, b, 128c+p]
        ek = (
            enc16[k * L_LOC : (k + 1) * L_LOC]
            .reshape(NBLK, LG, B, NCH, 128)
            .transpose(0, 4, 3, 1, 2)
            .reshape(NBLK * 128, NCH * 512)
        )
        in_maps.append(
            {
                "enc": np.ascontiguousarray(ek),
                "hbtt": hbtt16,
                "wfull": W16,
            }
        )
    return in_maps


def kernel(hidden, encoder_outputs, W, b):
    """Full inputs in, full [1, 1, L, B] output out; runs on 8 NeuronCores."""
    global last_results
    from concourse.bass_utils import run_bass_kernel_spmd

    nc = build_program()
    in_maps = make_in_maps(hidden, encoder_outputs, W, b)
    res = run_bass_kernel_spmd(
        nc,
        in_maps,
        list(range(NCORES)),
        trace=bool(os.environ.get("KERNEL_TRACE")),
    )
    last_results = res
    chunks = [res.results[k]["out"] for k in range(NCORES)]
    full = np.concatenate(chunks, axis=0).reshape(1, 1, L, B)
    return full.astype(np.float32)
